# revision 14
# baseline (speedup 1.0000x reference)
"""Depthwise Conv1d for 8 trn2 cores — channel-major device layout.

Sharding: batch-parallel (B == n_cores == 8). The host transposes each
batch element to channel-major [C, L] before dispatch and transposes the
[C, LOUT] result back afterwards (memory-bandwidth-bound numpy copies,
parallelized across batch elements). On device the depthwise conv is pure
shifted-slice arithmetic with no transposes, no PSUM, no TensorE work:

  xt[128c, W+3]  <- strided DMA from x_t (4-16KB contiguous runs/partition)
  acc  = Identity(xt[:, 0:W]*w0 + bias)   ACT, per-partition scale+bias
  acc  = (xt[:, k:k+W]*wk) + acc          DVE scalar_tensor_tensor, k=1,2,3
  out_t[cb, o0:o0+W] <- DMA acc

DVE is the critical path (3 fused multiply-add passes at fp32 1x mode,
~218us/core) slightly above the ~203us HBM-DMA floor; ACT carries tap 0.
GPSIMD stays idle on purpose: its SBUF access arbitrates an exclusive
port-pair lock against DVE 2-input ops and stalls both engines.
"""

import sys
from concurrent.futures import ThreadPoolExecutor

for _p in ("/opt/trn_rl_repo", "/root/.axon_site/_ro/trn_rl_repo"):
    if _p not in sys.path:
        sys.path.insert(0, _p)

import numpy as np

import concourse.bass as bass  # noqa: F401
import concourse.tile as tile
from concourse import bacc, mybir
from concourse.bass_utils import run_bass_kernel_spmd

F32 = mybir.dt.float32
MULT = mybir.AluOpType.mult
ADD = mybir.AluOpType.add
COPY = mybir.ActivationFunctionType.Copy
IDENT = mybir.ActivationFunctionType.Identity

B, L, C, K, PAD = 8, 4096, 2048, 4, 3
LOUT = L + 2 * PAD - K + 1  # 4099
NCB = C // 128  # 16 channel blocks
CHUNK = 2048
NCHUNK = 2  # chunk 0: [0,2048); chunk 1: [2048, 4099)
TAIL = LOUT - NCHUNK * CHUNK  # 3

# GPSIMD compute is intentionally unused: any GPSIMD tensor op arbitrates an
# exclusive SBUF port-pair lock against DVE 2-input ops, stalling both.
ACT_TAP1_CBS = 0


def _build_nc():
    nc = bacc.Bacc("TRN2", target_bir_lowering=False, num_devices=B)

    xt_d = nc.dram_tensor("xt", [C, L], F32, kind="ExternalInput")
    wt_d = nc.dram_tensor("wt", [128, NCB * K], F32, kind="ExternalInput")
    bt_d = nc.dram_tensor("bt", [128, NCB], F32, kind="ExternalInput")
    out_d = nc.dram_tensor("out", [C, LOUT], F32, kind="ExternalOutput")

    with tile.TileContext(nc) as tc:
        with (
            tc.tile_pool(name="const", bufs=1) as cpool,
            tc.tile_pool(name="xt", bufs=6) as xt_pool,
            tc.tile_pool(name="acc", bufs=5) as acc_pool,
        ):
            wt_sb = cpool.tile([128, NCB * K], F32)
            bt_sb = cpool.tile([128, NCB], F32)

            # tiny Identity activation up front so ACT_TABLE_LOAD happens
            # during the DMA ramp, not in front of the first real tap-0 op
            warm = cpool.tile([128, 1], F32)
            nc.vector.memset(warm[:], 0.0)
            nc.scalar.activation(
                out=warm[:], in_=warm[:], func=IDENT, scale=1.0
            )

            def load_xt(cb, o0, width):
                cs = slice(cb * 128, (cb + 1) * 128)
                xtw = width + PAD  # needs x cols [o0-3, o0+width)
                lo = o0 - PAD
                hi = min(o0 + width, L)
                xt = xt_pool.tile([128, xtw], F32, tag="xt")
                if o0 == 0:
                    nc.vector.memset(xt[:, 0:PAD], 0.0)
                    nc.sync.dma_start(
                        out=xt[:, PAD : PAD + hi], in_=xt_d[cs, 0:hi]
                    )
                else:
                    nc.sync.dma_start(
                        out=xt[:, 0 : hi - lo], in_=xt_d[cs, lo:hi]
                    )
                if hi - lo < xtw:
                    # zero-pad for virtual x rows beyond L
                    nc.vector.memset(xt[:, hi - lo : xtw], 0.0)
                return xt

            def emit_unit(cb, o0, width, xt=None):
                """Produce out[cb*128:(cb+1)*128, o0:o0+width]."""
                cs = slice(cb * 128, (cb + 1) * 128)
                if xt is None:
                    xt = load_xt(cb, o0, width)

                wk = lambda k: wt_sb[:, cb * K + k : cb * K + k + 1]

                acc = acc_pool.tile([128, width], F32, tag="acc")
                nc.scalar.activation(
                    out=acc[:],
                    in_=xt[:, 0:width],
                    func=IDENT,
                    scale=wk(0),
                    bias=bt_sb[:, cb : cb + 1],
                )
                for k in (1, 2, 3):
                    nc.vector.scalar_tensor_tensor(
                        out=acc[:],
                        in0=xt[:, k : k + width],
                        scalar=wk(k),
                        in1=acc[:],
                        op0=MULT,
                        op1=ADD,
                    )
                nc.sync.dma_start(out=out_d[cs, o0 : o0 + width], in_=acc[:])

            # small units first to prime the pipeline and last to drain it
            # quickly; full-L units in the middle. The first unit's input DMA
            # is issued before the (tiny) weight/bias DMAs to cut the ramp.
            units = [
                (0, 0, 128), (0, 128, 384), (0, 512, 512),
                (0, 1024, 1024), (0, 2048, 1024), (0, 3072, 1027),
                (1, 0, 2048), (1, 2048, 2051),
            ]
            units += [(cb, 0, LOUT) for cb in range(2, NCB - 1)]
            units += [(NCB - 1, 0, 2048), (NCB - 1, 2048, 1024),
                      (NCB - 1, 3072, 1027)]
            xt0 = load_xt(*units[0])
            nc.sync.dma_start(out=wt_sb[:], in_=wt_d[:])
            nc.sync.dma_start(out=bt_sb[:], in_=bt_d[:])
            emit_unit(*units[0], xt=xt0)
            for cb, o0, width in units[1:]:
                emit_unit(cb, o0, width)

    nc.compile()
    return nc


_NC_CACHE = None


def _get_nc():
    global _NC_CACHE
    if _NC_CACHE is None:
        _NC_CACHE = _build_nc()
    return _NC_CACHE


def _const_inputs(weight, bias):
    wt = np.ascontiguousarray(
        weight.astype(np.float32).reshape(NCB, 128, K).transpose(1, 0, 2)
    ).reshape(128, NCB * K)
    bt = np.ascontiguousarray(bias.astype(np.float32).reshape(NCB, 128).T)
    return wt, bt


def _in_maps(x, weight, bias):
    wt, bt = _const_inputs(weight, bias)
    with ThreadPoolExecutor(max_workers=8) as ex:
        xts = list(ex.map(lambda b: np.ascontiguousarray(x[b].T), range(B)))
    return [{"xt": xts[b], "wt": wt, "bt": bt} for b in range(B)]


def kernel(x, weight, bias):
    x = np.asarray(x)
    weight = np.asarray(weight)
    bias = np.asarray(bias)
    assert x.shape == (B, L, C) and weight.shape == (C, K) and bias.shape == (C,)
    nc = _get_nc()
    in_maps = _in_maps(x, weight, bias)
    res = run_bass_kernel_spmd(nc, in_maps, core_ids=list(range(B)))

    out = np.empty((B, LOUT, C), dtype=np.float32)
    with ThreadPoolExecutor(max_workers=8) as ex:
        list(
            ex.map(
                lambda b: np.copyto(out[b], res.results[b]["out"].T), range(B)
            )
        )
    return out


if __name__ == "__main__":
    rng = np.random.default_rng(0)
    x = rng.standard_normal((B, L, C), dtype=np.float32)
    w = (rng.standard_normal((C, K)) * 0.1).astype(np.float32)
    bias = (rng.standard_normal((C,)) * 0.1).astype(np.float32)
    out = kernel(x, w, bias)
    print("out", out.shape, out.dtype)


# revision 15
# speedup vs baseline: 1.0058x; 1.0058x over previous
"""Depthwise Conv1d for 8 trn2 cores — channel-major device layout.

Sharding: batch-parallel (B == n_cores == 8). The host transposes each
batch element to channel-major [C, L] before dispatch and transposes the
[C, LOUT] result back afterwards (memory-bandwidth-bound numpy copies,
parallelized across batch elements). On device the depthwise conv is pure
shifted-slice arithmetic with no transposes, no PSUM, no TensorE work:

  xt[128c, W+3]  <- strided DMA from x_t (4-16KB contiguous runs/partition)
  acc  = Identity(xt[:, 0:W]*w0 + bias)   ACT, per-partition scale+bias
  acc  = (xt[:, k:k+W]*wk) + acc          DVE scalar_tensor_tensor, k=1,2,3
  out_t[cb, o0:o0+W] <- DMA acc

DVE is the critical path (3 fused multiply-add passes at fp32 1x mode,
~218us/core) slightly above the ~203us HBM-DMA floor; ACT carries tap 0.
GPSIMD stays idle on purpose: its SBUF access arbitrates an exclusive
port-pair lock against DVE 2-input ops and stalls both engines.
"""

import sys
from concurrent.futures import ThreadPoolExecutor

for _p in ("/opt/trn_rl_repo", "/root/.axon_site/_ro/trn_rl_repo"):
    if _p not in sys.path:
        sys.path.insert(0, _p)

import numpy as np

import concourse.bass as bass  # noqa: F401
import concourse.tile as tile
from concourse import bacc, mybir
from concourse.bass_utils import run_bass_kernel_spmd

F32 = mybir.dt.float32
MULT = mybir.AluOpType.mult
ADD = mybir.AluOpType.add
COPY = mybir.ActivationFunctionType.Copy
IDENT = mybir.ActivationFunctionType.Identity

B, L, C, K, PAD = 8, 4096, 2048, 4, 3
LOUT = L + 2 * PAD - K + 1  # 4099
NCB = C // 128  # 16 channel blocks
CHUNK = 2048
NCHUNK = 2  # chunk 0: [0,2048); chunk 1: [2048, 4099)
TAIL = LOUT - NCHUNK * CHUNK  # 3

# GPSIMD compute is intentionally unused: any GPSIMD tensor op arbitrates an
# exclusive SBUF port-pair lock against DVE 2-input ops, stalling both.
ACT_TAP1_CBS = 0


def _build_nc():
    nc = bacc.Bacc("TRN2", target_bir_lowering=False, num_devices=B)

    xt_d = nc.dram_tensor("xt", [C, L], F32, kind="ExternalInput")
    wt_d = nc.dram_tensor("wt", [128, NCB * K], F32, kind="ExternalInput")
    bt_d = nc.dram_tensor("bt", [128, NCB], F32, kind="ExternalInput")
    out_d = nc.dram_tensor("out", [C, LOUT], F32, kind="ExternalOutput")

    with tile.TileContext(nc) as tc:
        with (
            tc.tile_pool(name="const", bufs=1) as cpool,
            tc.tile_pool(name="xt", bufs=6) as xt_pool,
            tc.tile_pool(name="acc", bufs=5) as acc_pool,
        ):
            wt_sb = cpool.tile([128, NCB * K], F32)
            bt_sb = cpool.tile([128, NCB], F32)

            # tiny Identity activation up front so ACT_TABLE_LOAD happens
            # during the DMA ramp, not in front of the first real tap-0 op
            warm = cpool.tile([128, 1], F32)
            nc.vector.memset(warm[:], 0.0)
            nc.scalar.activation(
                out=warm[:], in_=warm[:], func=IDENT, scale=1.0
            )

            def load_xt(cb, o0, width):
                cs = slice(cb * 128, (cb + 1) * 128)
                xtw = width + PAD  # needs x cols [o0-3, o0+width)
                lo = o0 - PAD
                hi = min(o0 + width, L)
                xt = xt_pool.tile([128, xtw], F32, tag="xt")
                if o0 == 0:
                    nc.vector.memset(xt[:, 0:PAD], 0.0)
                    nc.sync.dma_start(
                        out=xt[:, PAD : PAD + hi], in_=xt_d[cs, 0:hi]
                    )
                else:
                    nc.sync.dma_start(
                        out=xt[:, 0 : hi - lo], in_=xt_d[cs, lo:hi]
                    )
                if hi - lo < xtw:
                    # zero-pad for virtual x rows beyond L
                    nc.vector.memset(xt[:, hi - lo : xtw], 0.0)
                return xt

            def emit_unit(cb, o0, width, xt=None):
                """Produce out[cb*128:(cb+1)*128, o0:o0+width]."""
                cs = slice(cb * 128, (cb + 1) * 128)
                if xt is None:
                    xt = load_xt(cb, o0, width)

                wk = lambda k: wt_sb[:, cb * K + k : cb * K + k + 1]

                acc = acc_pool.tile([128, width], F32, tag="acc")
                nc.scalar.activation(
                    out=acc[:],
                    in_=xt[:, 0:width],
                    func=IDENT,
                    scale=wk(0),
                    bias=bt_sb[:, cb : cb + 1],
                )
                for k in (1, 2, 3):
                    nc.vector.scalar_tensor_tensor(
                        out=acc[:],
                        in0=xt[:, k : k + width],
                        scalar=wk(k),
                        in1=acc[:],
                        op0=MULT,
                        op1=ADD,
                    )
                nc.sync.dma_start(out=out_d[cs, o0 : o0 + width], in_=acc[:])

            # small units first to prime the pipeline and last to drain it
            # quickly; full-L units in the middle. The first unit's input DMA
            # is issued before the (tiny) weight/bias DMAs to cut the ramp.
            units = [
                (0, 0, 128), (0, 128, 384), (0, 512, 512),
                (0, 1024, 1024), (0, 2048, 1024), (0, 3072, 1027),
                (1, 0, 2048), (1, 2048, 2051),
            ]
            units += [(cb, 0, LOUT) for cb in range(2, NCB - 1)]
            units += [(NCB - 1, 0, 2048), (NCB - 1, 2048, 2051)]
            xt0 = load_xt(*units[0])
            nc.sync.dma_start(out=wt_sb[:], in_=wt_d[:])
            nc.sync.dma_start(out=bt_sb[:], in_=bt_d[:])
            emit_unit(*units[0], xt=xt0)
            for cb, o0, width in units[1:]:
                emit_unit(cb, o0, width)

    nc.compile()
    return nc


_NC_CACHE = None


def _get_nc():
    global _NC_CACHE
    if _NC_CACHE is None:
        _NC_CACHE = _build_nc()
    return _NC_CACHE


def _const_inputs(weight, bias):
    wt = np.ascontiguousarray(
        weight.astype(np.float32).reshape(NCB, 128, K).transpose(1, 0, 2)
    ).reshape(128, NCB * K)
    bt = np.ascontiguousarray(bias.astype(np.float32).reshape(NCB, 128).T)
    return wt, bt


def _in_maps(x, weight, bias):
    wt, bt = _const_inputs(weight, bias)
    with ThreadPoolExecutor(max_workers=8) as ex:
        xts = list(ex.map(lambda b: np.ascontiguousarray(x[b].T), range(B)))
    return [{"xt": xts[b], "wt": wt, "bt": bt} for b in range(B)]


def kernel(x, weight, bias):
    x = np.asarray(x)
    weight = np.asarray(weight)
    bias = np.asarray(bias)
    assert x.shape == (B, L, C) and weight.shape == (C, K) and bias.shape == (C,)
    nc = _get_nc()
    in_maps = _in_maps(x, weight, bias)
    res = run_bass_kernel_spmd(nc, in_maps, core_ids=list(range(B)))

    out = np.empty((B, LOUT, C), dtype=np.float32)
    with ThreadPoolExecutor(max_workers=8) as ex:
        list(
            ex.map(
                lambda b: np.copyto(out[b], res.results[b]["out"].T), range(B)
            )
        )
    return out


if __name__ == "__main__":
    rng = np.random.default_rng(0)
    x = rng.standard_normal((B, L, C), dtype=np.float32)
    w = (rng.standard_normal((C, K)) * 0.1).astype(np.float32)
    bias = (rng.standard_normal((C,)) * 0.1).astype(np.float32)
    out = kernel(x, w, bias)
    print("out", out.shape, out.dtype)
